# revision 53
# baseline (speedup 1.0000x reference)
"""Causal self-attention on 8 Trainium2 NeuronCores.

Problem: B=2, T=2048, C=1024, 16 heads x 64 dim, fp32.

Sharding: tensor-parallel over heads x data-parallel over batch.
Each core owns one batch element (cores 0-3 -> b=0, 4-7 -> b=1) and a
group of 4 consecutive heads. Each core computes:
  - QKV projection for its 4 heads (producing qT/kT transposed, V natural)
  - causal attention for its 4 heads (scores kept transposed: ST[tk, tq])
  - partial output projection (its heads' rows of w_proj)
The host sums the 4 partial projections per batch and adds b_proj.

All inputs/outputs bf16 (halves DMA; matmuls run bf16 = full PE rate).

Fused pipeline: the program is one stream per slab s of 512 queries:
  QKV(0); for s: ATT(s, pair0), ATT(s, pair1) with QKV(s+1)/OP(s-1)
  matmul units interleaved as PE fillers inside the tk loop, so the PE
  stays busy while ACT computes exp(ST).
Softmax denominators come from a ones-column appended to V (row 64 of
the PV psum accumulator).

Optimizations vs the first working kernel (166.9us -> ~152us):
  - Startup: the critical wqk(pair0-cols)/x0/wv loads stream in
    per-k-pair chunks on the SP queue so the QKV matmuls chase the
    DMA; all other loads are chained behind them with 1-element
    gpsimd dep-copies (HW DGE queues transfer concurrently and would
    otherwise halve the critical stream's bandwidth); small tensors
    ride the ACT hwdge queue.  Dummy matmuls on a scratch tile keep
    the PE busy through the load wait so the HAM clock-gate is warm
    (2.4 GHz) when real work arrives.
  - Drain: one [65,512] psum->sbuf copy per head frees the psy bank
    (hp0 on DVE, hp1 on ACT, in parallel); the reciprocal / partition-
    broadcast / normalize-multiply run as "lazy units" popped one per
    tk step inside the NEXT att call, spread so the muls pop >=2 steps
    after their gpsimd broadcasts and never head-block the strict-FIFO
    DVE queue (which previously stalled psum frees -> PE).
  - Tail: att(3,1) normalizes inline -- sm copies on ACT, per-head
    reciprocal chains on DVE, PE outer-product broadcast
    (ones[1,64].T @ rec[1,512] -> psum) instead of the ~1us gpsimd
    broadcast; dummy matmuls bridge the chain latency so HAM stays
    warm; op12's pair-0 matmuls pre-issue during the chain; tail op
    units alternate cast engines (ACT/DVE), psum rings (misc/psy) and
    DMA issue queues (SP/ACT) to avoid serializing on any one of them.
  - Explicit per-att filler lists keep every att's first PV covered
    with independent PE work (slab-0 v-blocks inside att(0,0), qk of
    the next slab inside att(s,1), op units late enough that the
    lazily-normalized yT they read is already emitted).

Device layouts (per core, DRAM):
  xT   [1024, 2048] bf16  x[b] transposed (channels on partitions)
  wqk  [1024, 512]  bf16  cols: q(h0)|q(h1)|k(h0)|k(h1)|q(h2)|q(h3)|k(h2)|k(h3)
  wv   [1024, 256]  bf16  v cols of the 4 heads
  wo   [256, 1024]  bf16  w_proj rows of the 4 heads
  bqk  [4, 128]     f32   rows: pair0-q, pair0-k, pair1-q, pair1-k biases
  bv   [256]        f32   v bias of the 4 heads
  mask [128, 128]   bf16  mask[i,j] = 1 if i<=j else 0 (tk<=tq keep)
  out  [2048, 1024] bf16  partial (pre-bias) output projection
"""

import numpy as np

B, T, C = 2, 2048, 1024
NH, DH = 16, 64
NCORES = 8
HPC = 4  # heads per core
P = 128
CK = C // P  # 8 contraction tiles over channels
NT = T // P  # 16 token tiles
SLAB = 512
NSL = T // SLAB  # 4 tq slabs

import os
WARMUP = int(os.environ.get("K_WARMUP", "52"))
ACT_DMA = int(os.environ.get("K_ACT_DMA", "1"))
FINE_LOADS = int(os.environ.get("K_FINE_LOADS", "1"))

_CACHE = {}


def _build_program():
    from contextlib import ExitStack

    import concourse.bacc as bacc
    import concourse.bass as bass
    import concourse.tile as tile
    from concourse import mybir

    f32 = mybir.dt.float32
    bf16 = mybir.dt.bfloat16
    AF = mybir.ActivationFunctionType

    nc = bacc.Bacc(
        "TRN2", target_bir_lowering=False, debug=False, num_devices=NCORES
    )

    xT = nc.dram_tensor("xT", [C, T], bf16, kind="ExternalInput").ap()
    wqk = nc.dram_tensor("wqk", [C, 4 * P], bf16, kind="ExternalInput").ap()
    wv = nc.dram_tensor("wv", [C, HPC * DH], bf16, kind="ExternalInput").ap()
    wo = nc.dram_tensor("wo", [HPC * DH, C], bf16, kind="ExternalInput").ap()
    bqk = nc.dram_tensor("bqk", [4, P], f32, kind="ExternalInput").ap()
    bv = nc.dram_tensor("bv", [HPC * DH], f32, kind="ExternalInput").ap()
    mask = nc.dram_tensor("mask", [P, P], bf16, kind="ExternalInput").ap()
    out = nc.dram_tensor("out", [T, C], bf16, kind="ExternalOutput").ap()

    with tile.TileContext(nc) as tc, ExitStack() as ctx:
        const = ctx.enter_context(tc.tile_pool(name="const", bufs=1))
        # PSUM (8 banks of [128,512]f32): pp 2x2 + psy 2x1 + misc 2x1
        ppp = ctx.enter_context(tc.tile_pool(name="ppp", bufs=2, space="PSUM"))
        psyp = ctx.enter_context(tc.tile_pool(name="psyp", bufs=2, space="PSUM"))
        miscp = ctx.enter_context(tc.tile_pool(name="miscp", bufs=2, space="PSUM"))
        expp = ctx.enter_context(tc.tile_pool(name="expp", bufs=4))
        recp = ctx.enter_context(tc.tile_pool(name="recp", bufs=4))
        outp = ctx.enter_context(tc.tile_pool(name="outp", bufs=3))

        x_sb = [
            const.tile([P, CK, SLAB], bf16, name=f"x{s}") for s in range(NSL)
        ]
        wqk_sb = const.tile([P, CK, 4 * P], bf16, name="wqk_sb")
        wv_sb = const.tile([P, CK, HPC * DH], bf16, name="wv_sb")
        wo_sb = const.tile([P, 2, C], bf16, name="wo_sb")
        bqk_sb = const.tile([P, 4], f32, name="bqk_sb")
        bv_sb = const.tile([P, HPC, DH], f32, name="bv_sb")
        mask_sb = const.tile([P, P], bf16, name="mask_sb")
        warm_sb = const.tile([P, 2 * P], bf16, name="warm_sb")
        ones_sb = const.tile([1, DH], bf16, name="ones_sb")
        v_sb = [
            const.tile([P, 4, HPC, DH + 1], bf16, name=f"v_sb{s}")
            for s in range(NSL)
        ]
        qT = [
            [const.tile([P, SLAB], bf16, name=f"qT{p}_{s}") for s in range(NSL)]
            for p in range(2)
        ]
        kT = [
            [const.tile([P, SLAB], bf16, name=f"kT{p}_{s}") for s in range(NSL)]
            for p in range(2)
        ]
        yT = [
            [const.tile([P, SLAB], bf16, name=f"yT{p}_{s}") for s in range(NSL)]
            for p in range(2)
        ]

        # --- loads -------------------------------------------------------
        # SP queue carries the startup-critical wqk/x0 stream in per-k-pair
        # chunks (the first QKV matmuls chase the DMA), then x slabs 1-3.
        # The ACT hwdge queue carries everything else in parallel.
        wqkv_ = wqk.rearrange("(k p) n -> p k n", p=P)
        xTv = xT.rearrange("(k p) t -> p k t", p=P)
        eng2 = nc.scalar if ACT_DMA else nc.sync
        if FINE_LOADS:
            # critical stream: per-k-pair chunks so QKV matmuls chase
            # DMA.  Only pair-0's wqk columns (0:256) are critical —
            # pair-1's (qk blocks 2/3) are needed ~8us later.
            for k2 in range(0, CK, 2):
                nc.sync.dma_start(
                    out=wqk_sb[:, k2 : k2 + 2, 0 : 2 * P],
                    in_=wqkv_[:, k2 : k2 + 2, 0 : 2 * P],
                )
                nc.sync.dma_start(
                    out=x_sb[0][:, k2 : k2 + 2, :],
                    in_=xTv[:, k2 : k2 + 2, 0:SLAB],
                )
        else:
            h = CK // 2
            nc.sync.dma_start(out=wqk_sb[:, :h, 0 : 2 * P], in_=wqkv_[:, :h, 0 : 2 * P])
            nc.sync.dma_start(out=x_sb[0][:, :h, :], in_=xTv[:, :h, 0:SLAB])
            nc.sync.dma_start(out=wqk_sb[:, h:, 0 : 2 * P], in_=wqkv_[:, h:, 0 : 2 * P])
            nc.sync.dma_start(out=x_sb[0][:, h:, :], in_=xTv[:, h:, 0:SLAB])
        nc.sync.dma_start(
            out=wv_sb[:], in_=wv.rearrange("(k p) n -> p k n", p=P)
        )
        # Cascade the non-critical loads: a 1-element gpsimd copy makes
        # each DMA wait for the previous one, so the startup-critical
        # wqk/x0/wv stream gets the DMA engines to itself.  (HW DGE
        # queues transfer concurrently; without this, the rest would
        # steal ~half the bandwidth exactly when QKV(0) is data-starved.)
        # The dep copies MUST NOT run on DVE/ACT: they wait on DMA
        # completion semaphores at the head of a strict-FIFO queue and
        # would block mask-muls / exps behind them.  GpSimd only carries
        # the slack-tolerant lazy broadcasts.
        def casc_dep(dst_ap, src_ap):
            nc.gpsimd.partition_broadcast(out_ap=dst_ap, in_ap=src_ap)

        casc_dep(wqk_sb[0:1, 0, 2 * P : 2 * P + 1], wv_sb[0:1, 0, 0:1])
        nc.sync.dma_start(
            out=wqk_sb[:, :, 2 * P :], in_=wqkv_[:, :, 2 * P :]
        )
        # x1 runs in parallel with wqkB (both ~0.5MB, both needed by
        # att(0,1)'s qk(1,*) fillers at about the same time)
        casc_dep(x_sb[1][0:1, 0, 0:1], wv_sb[0:1, 0, 1:2])
        h = CK // 2
        nc.sync.dma_start(out=x_sb[1][:, :h, :], in_=xTv[:, :h, SLAB : 2 * SLAB])
        nc.sync.dma_start(out=x_sb[1][:, h:, :], in_=xTv[:, h:, SLAB : 2 * SLAB])
        cascade = [x_sb[2], wo_sb, x_sb[3]]
        prev_dep = x_sb[1][0:1, CK - 1, SLAB - 1 : SLAB]
        for tgt in cascade:
            if tgt is wo_sb:
                casc_dep(wo_sb[0:1, 0, 0:1], prev_dep)
                eng2.dma_start(
                    out=wo_sb[:], in_=wo.rearrange("(r p) n -> p r n", p=P)
                )
                prev_dep = wo_sb[0:1, 1, C - 1 : C]
            else:
                s = x_sb.index(tgt)
                h = CK // 2
                casc_dep(tgt[0:1, 0, 0:1], prev_dep)
                nc.sync.dma_start(
                    out=tgt[:, :h, :],
                    in_=xTv[:, :h, s * SLAB : (s + 1) * SLAB],
                )
                nc.sync.dma_start(
                    out=tgt[:, h:, :],
                    in_=xTv[:, h:, s * SLAB : (s + 1) * SLAB],
                )
                prev_dep = tgt[0:1, CK - 1, SLAB - 1 : SLAB]
        # small/late tensors on the ACT hwdge queue (ACT boots ~7us; these
        # must not steal DMA bandwidth from the critical wqk/x0 stream)
        eng2.dma_start(out=bqk_sb[:], in_=bqk.rearrange("r p -> p r"))
        eng2.dma_start(out=mask_sb[:], in_=mask)
        bv_bcast = bass.AP(
            tensor=bv.tensor,
            offset=bv.offset,
            ap=[[0, P], *bv.rearrange("(h d) -> h d", d=DH).ap],
        )
        eng2.dma_start(out=bv_sb[:], in_=bv_bcast)
        for s in range(NSL):
            nc.vector.memset(v_sb[s][:, :, :, DH : DH + 1], 1.0)
        nc.vector.memset(warm_sb[:], 0.125)
        nc.vector.memset(ones_sb[:], 1.0)

        # --- PE warmup: dummy matmuls on the scratch tile keep the PE
        # busy through the load wait so HAM un-throttles (K=8/8) before
        # the first real matmul (~3.4us of sustained activity needed).
        if WARMUP:
            wps = miscp.tile([P, SLAB], f32, name="wps", tag="m")
            for _ in range(WARMUP):
                nc.tensor.matmul(
                    wps[:, 0:P],
                    lhsT=warm_sb[:, 0:P],
                    rhs=warm_sb[:, P : 2 * P],
                    start=True,
                    stop=True,
                )

        # --- work units ---
        def qk_block(s, blk):
            """One q/k column block of QKV(s): 8 chained MMs + ACT bias."""
            p, qk = divmod(blk, 2)
            dst = qT[p][s] if qk == 0 else kT[p][s]
            ps = miscp.tile([P, SLAB], f32, name="ps_qkv", tag="m")
            for k in range(CK):
                nc.tensor.matmul(
                    ps[:],
                    lhsT=wqk_sb[:, k, blk * P : (blk + 1) * P],
                    rhs=x_sb[s][:, k, :],
                    start=(k == 0),
                    stop=(k == CK - 1),
                )
            nc.scalar.activation(
                out=dst[:],
                in_=ps[:],
                func=AF.Identity,
                bias=bqk_sb[:, blk : blk + 1],
                scale=1.0,
            )

        def v_block(s, tt):
            """V for token tile 4s+tt: 8 chained MMs + DVE bias add."""
            ps = miscp.tile([P, SLAB], f32, name="ps_v", tag="m")
            for k in range(CK):
                nc.tensor.matmul(
                    ps[:, : HPC * DH],
                    lhsT=x_sb[s][:, k, tt * P : (tt + 1) * P],
                    rhs=wv_sb[:, k, :],
                    start=(k == 0),
                    stop=(k == CK - 1),
                )
            nc.vector.tensor_add(
                out=v_sb[s][:, tt, :, 0:DH],
                in0=ps[:, : HPC * DH].rearrange("p (h d) -> p h d", d=DH),
                in1=bv_sb[:],
            )

        def op_unit(t, act_cast=False, split_dma=False, psy_pool=False):
            """Output projection for token tile t + drain + DMA."""
            ob = outp.tile([P, C], bf16, name="ob", tag="ob")
            for ns in range(2):
                if psy_pool:  # tail only: psy banks are free, doubles
                    ps = psyp.tile([P, SLAB], f32, name="psot", tag="psy")
                else:
                    ps = miscp.tile([P, SLAB], f32, name="pso", tag="m")
                for p in range(2):
                    nc.tensor.matmul(
                        ps[:],
                        lhsT=yT[p][t // 4][:, (t % 4) * P : (t % 4 + 1) * P],
                        rhs=wo_sb[:, p, ns * SLAB : (ns + 1) * SLAB],
                        start=(p == 0),
                        stop=(p == 1),
                    )
                if act_cast:
                    nc.scalar.copy(
                        out=ob[:, ns * SLAB : (ns + 1) * SLAB], in_=ps[:]
                    )
                else:
                    nc.vector.tensor_copy(
                        out=ob[:, ns * SLAB : (ns + 1) * SLAB], in_=ps[:]
                    )
                # tail ops split DMA issue across SP/ACT queues (12 back
                # -to-back issues at 565ns each would serialize ~7us)
                deng = nc.scalar if (split_dma and ns == 1) else nc.sync
                deng.dma_start(
                    out=out[t * P : (t + 1) * P, ns * SLAB : (ns + 1) * SLAB],
                    in_=ob[:, ns * SLAB : (ns + 1) * SLAB],
                )

        lazy = []  # deferred normalize sub-units, popped 1/tk-step
        noop_u = lambda: None

        def att(s, p, fillers, post=(), last=False):
            """Causal attention for head pair p over tq slab s.

            Pops one PE filler + one lazy normalize unit per tk step.
            """
            ntk = 4 * s + 4
            psy = [
                psyp.tile([P, SLAB], f32, name=f"psy{hp}", tag="psy")
                for hp in range(2)
            ]

            def off_of(tk):
                d = tk - 4 * s
                return d * P if d >= 0 else 0

            pend = {}
            exd = {}

            def st(tk):
                off = off_of(tk)
                pp = ppp.tile([P, 2 * SLAB], f32, name="pp", tag="pp")
                for hp in range(2):
                    nc.tensor.matmul(
                        pp[:, hp * SLAB + off : (hp + 1) * SLAB],
                        lhsT=kT[p][tk // 4][
                            hp * DH : (hp + 1) * DH,
                            (tk % 4) * P : (tk % 4 + 1) * P,
                        ],
                        rhs=qT[p][s][hp * DH : (hp + 1) * DH, off:],
                        start=True,
                        stop=True,
                    )
                pend[tk] = pp

            def do_exp(tk):
                off = off_of(tk)
                pp = pend.pop(tk)
                ex = expp.tile([P, 2 * SLAB], bf16, name="ex", tag="ex")
                ppv = pp[:].rearrange("q (h n) -> q h n", h=2)[:, :, off:]
                exv = ex[:].rearrange("q (h n) -> q h n", h=2)[:, :, off:]
                nc.scalar.activation(
                    out=exv,
                    in_=ppv,
                    func=AF.Exp,
                    scale=float(1.0 / np.sqrt(DH)),
                )
                if tk - 4 * s >= 0:
                    exm = ex[:].rearrange("q (h n) -> q h n", h=2)[
                        :, :, off : off + P
                    ]
                    mask2 = bass.AP(
                        tensor=mask_sb[:].tensor,
                        offset=mask_sb[:].offset,
                        ap=[mask_sb[:].ap[0], [0, 2], mask_sb[:].ap[1]],
                    )
                    nc.vector.tensor_mul(out=exm, in0=exm, in1=mask2)
                exd[tk] = ex

            st(0)
            if ntk > 1:
                st(1)
            do_exp(0)
            for tk in range(ntk):
                off = off_of(tk)
                if tk + 2 < ntk:
                    st(tk + 2)
                if lazy:
                    lazy.pop(0)()
                if fillers:
                    fillers.pop(0)()
                if tk + 1 < ntk:
                    do_exp(tk + 1)
                ex = exd.pop(tk)
                for hp in range(2):
                    nc.tensor.matmul(
                        psy[hp][0 : DH + 1, off:],
                        lhsT=v_sb[tk // 4][:, tk % 4, 2 * p + hp, :],
                        rhs=ex[:, hp * SLAB + off : (hp + 1) * SLAB],
                        start=(tk == 0),
                        stop=(tk == ntk - 1),
                    )
            if last:
                # tail fast path.  The denominator rows are pulled from
                # PSUM immediately (sm copies), reciprocals cast to bf16
                # feed a PE outer-product broadcast (ones[1,64].T @ rec)
                # while dummy+post matmuls keep the PE warm; normalize
                # muls land before post[1] so the final OP tiles start
                # right after.  Post casts go to ACT (DVE is the tail
                # critical path).
                post = list(post)
                # dummies bridge PE to the post matmuls (whose psum
                # slots wait on casts); pp ring is free here
                wps2 = ppp.tile([P, 2 * SLAB], f32, name="wps2", tag="pp")
                for _ in range(16):
                    nc.tensor.matmul(
                        wps2[:, 0 : 2 * P],
                        lhsT=warm_sb[:, 0:P],
                        rhs=warm_sb[:],
                        start=True,
                        stop=True,
                    )
                # per-head chains: sm on ACT, recip+cast on DVE, so the
                # hp0 broadcast matmul can start ~3 ops after last PV
                recs = []
                for hp in range(2):
                    sm = recp.tile([1, SLAB], f32, name="smL", tag="smL")
                    nc.scalar.copy(out=sm[:], in_=psy[hp][DH : DH + 1, :])
                    recf = recp.tile([1, SLAB], f32, name="recf", tag="recf")
                    nc.vector.reciprocal_approx_fast(
                        out=recf[:], in_=sm[:]
                    )
                    rec = recp.tile([1, SLAB], bf16, name="recb", tag="recb")
                    nc.vector.tensor_copy(out=rec[:], in_=recf[:])
                    recs.append(rec)
                yfull = []
                for hp in range(2):
                    yf = recp.tile([DH, SLAB], f32, name="yfL", tag="yfL")
                    if hp == 0:
                        nc.vector.tensor_copy(out=yf[:], in_=psy[hp][0:DH, :])
                    else:
                        nc.scalar.copy(out=yf[:], in_=psy[hp][0:DH, :])
                    yfull.append(yf)
                # post MMs cover the DVE chain; second unit on the psy
                # ring (free after the sm/yf reads) so the two units
                # don't serialize on the 2-slot misc ring
                if post:
                    post[0](act_cast=True, split_dma=True)
                for f in post[1:]:
                    f(act_cast=True, split_dma=True, psy_pool=True)
                # pre-issue op12's pair-0 matmuls (yT[0][3] is already
                # normalized via the lazy units) during the chain wait
                pre_ps = []
                for ns in range(2):
                    ps = miscp.tile([P, SLAB], f32, name="pso", tag="m")
                    nc.tensor.matmul(
                        ps[:],
                        lhsT=yT[0][3][:, 0:P],
                        rhs=wo_sb[:, 0, ns * SLAB : (ns + 1) * SLAB],
                        start=True,
                        stop=False,
                    )
                    pre_ps.append(ps)
                rbs = []
                for hp in range(2):
                    rb = ppp.tile([P, 2 * SLAB], f32, name="rbL", tag="pp")
                    nc.tensor.matmul(
                        rb[0:DH, 0:SLAB],
                        lhsT=ones_sb[:],
                        rhs=recs[hp][:],
                        start=True,
                        stop=True,
                    )
                    rbs.append(rb)
                for hp in range(2):
                    nc.vector.tensor_mul(
                        out=yT[p][s][hp * DH : (hp + 1) * DH, :],
                        in0=yfull[hp][:],
                        in1=rbs[hp][0:DH, 0:SLAB],
                    )
                # finish op12: pair-1 matmuls + drains
                ob = outp.tile([P, C], bf16, name="ob", tag="ob")
                for ns in range(2):
                    nc.tensor.matmul(
                        pre_ps[ns][:],
                        lhsT=yT[1][3][:, 0:P],
                        rhs=wo_sb[:, 1, ns * SLAB : (ns + 1) * SLAB],
                        start=False,
                        stop=True,
                    )
                    if ns == 0:
                        nc.scalar.copy(
                            out=ob[:, ns * SLAB : (ns + 1) * SLAB],
                            in_=pre_ps[ns][:],
                        )
                    else:
                        nc.vector.tensor_copy(
                            out=ob[:, ns * SLAB : (ns + 1) * SLAB],
                            in_=pre_ps[ns][:],
                        )
                    deng = nc.scalar if ns == 1 else nc.sync
                    deng.dma_start(
                        out=out[12 * P : 13 * P, ns * SLAB : (ns + 1) * SLAB],
                        in_=ob[:, ns * SLAB : (ns + 1) * SLAB],
                    )
                return
            # drain: ONE [65,512] copy per head frees the psy bank fast
            # (hp0 on DVE, hp1 on ACT so they land in parallel).
            yfull = []
            for hp in range(2):
                yf = recp.tile([DH + 1, SLAB], f32, name="yf", tag="yf")
                if hp == 0:
                    nc.vector.tensor_copy(out=yf[:], in_=psy[hp][0 : DH + 1, :])
                else:
                    nc.scalar.copy(out=yf[:], in_=psy[hp][0 : DH + 1, :])
                yfull.append(yf)
            for f in post:  # PE work emitted after the drain copies
                f()
            if True:
                # deferred: recip+bcast then mul, popped in the next att
                def u_rb(hp):
                    def run():
                        # copy row 64 to partition 0 first: DVE table ops
                        # (recip) cannot shift partitions on HW
                        sm = recp.tile([1, SLAB], f32, name="sm", tag="sm")
                        nc.vector.tensor_copy(
                            out=sm[:], in_=yfull[hp][DH : DH + 1, :]
                        )
                        rec = recp.tile([1, SLAB], f32, name="rec", tag="rec")
                        nc.vector.reciprocal_approx_fast(
                            out=rec[:], in_=sm[:]
                        )
                        rb = recp.tile([DH, SLAB], f32, name="rb", tag="rb")
                        nc.gpsimd.partition_broadcast(
                            out_ap=rb[:], in_ap=rec[:]
                        )
                        yfull.append(rb)  # stash for u_mul

                    return run

                def u_mul(hp):
                    def run():
                        rb = yfull[2 + hp]
                        nc.vector.tensor_mul(
                            out=yT[p][s][hp * DH : (hp + 1) * DH, :],
                            in0=yfull[hp][0:DH, :],
                            in1=rb[:],
                        )

                    return run

                # spread pops: muls land >=2 steps after their gpsimd
                # broadcasts so they never head-block the DVE FIFO
                lazy.extend(
                    [u_rb(0), u_rb(1), noop_u, u_mul(0), u_mul(1), noop_u]
                )

        # --- fused schedule (explicit per-att filler lists) ---
        # Constraints encoded below:
        #  - v(0,*) interleave INTO att(0,0) so attention starts as soon
        #    as qk(0,0/1) + v(0,0) land (startup is DMA-paced).
        #  - qk(s,2/3) must be emitted before att(s,1) starts (st reads
        #    them), so qk(0,3) is emitted between the slab-0 atts.
        #  - every att(s,1) list starts with real PE work so its first
        #    PV (waiting on exp(0)) never idles the PE.
        #  - op units for slab s-1 pop only at step >= 4 of att(s,0)
        #    (the lazy normalize muls for yT[1][s-1] pop at steps 2-3).
        def qk(s, b):
            return lambda: qk_block(s, b)

        def vb(s, t):
            return lambda: v_block(s, t)

        def op(t):
            return lambda **kw: op_unit(t, **kw)

        noop = lambda: None
        qk_block(0, 0)
        qk_block(0, 1)
        v_block(0, 0)
        att(0, 0, [vb(0, 1), vb(0, 2), vb(0, 3), qk(0, 2)])
        qk_block(0, 3)
        att(0, 1, [qk(1, 0), qk(1, 1), qk(1, 2), qk(1, 3)])
        att(1, 0, [vb(1, 0), vb(1, 1), vb(1, 2), vb(1, 3), noop, noop,
                   op(0), op(1)])
        att(1, 1, [qk(2, 0), qk(2, 1), qk(2, 2), qk(2, 3), vb(2, 0),
                   vb(2, 1)], post=[op(2), op(3)])
        att(2, 0, [vb(2, 2), vb(2, 3), noop, noop, noop, op(4), op(5)])
        att(2, 1, [qk(3, 0), qk(3, 1), qk(3, 2), qk(3, 3), vb(3, 0),
                   vb(3, 1)], post=[op(6), op(7)])
        att(3, 0, [vb(3, 2), vb(3, 3), noop, noop, noop, op(8)])
        att(3, 1, [op(9)], post=[op(10), op(11)], last=True)  # op12 inlined
        for t in range(13, NT):
            op_unit(
                t,
                act_cast=(t % 2 == 0),
                split_dma=True,
                psy_pool=(t % 2 == 1),
            )

    nc.compile()
    return nc


def get_program():
    if "nc" not in _CACHE:
        _CACHE["nc"] = _build_program()
    return _CACHE["nc"]


def make_core_inputs(x, w_attn, b_attn, w_proj, core):
    """Host-side shard preparation for one core."""
    import ml_dtypes

    bf16 = ml_dtypes.bfloat16
    b = core // 4
    g = core % 4
    heads = [4 * g + i for i in range(HPC)]

    xT = np.ascontiguousarray(np.asarray(x[b], np.float32).T.astype(bf16))

    def qcols(h):
        return w_attn[:, h * DH : (h + 1) * DH]

    def kcols(h):
        return w_attn[:, C + h * DH : C + (h + 1) * DH]

    def vcols(h):
        return w_attn[:, 2 * C + h * DH : 2 * C + (h + 1) * DH]

    h0, h1, h2, h3 = heads
    wqk = np.ascontiguousarray(
        np.concatenate(
            [qcols(h0), qcols(h1), kcols(h0), kcols(h1),
             qcols(h2), qcols(h3), kcols(h2), kcols(h3)],
            axis=1,
        ).astype(bf16)
    )
    wv = np.ascontiguousarray(
        np.concatenate([vcols(h) for h in heads], axis=1).astype(bf16)
    )
    bqk = np.stack(
        [
            np.concatenate([b_attn[h0 * DH : (h0 + 1) * DH], b_attn[h1 * DH : (h1 + 1) * DH]]),
            np.concatenate([b_attn[C + h0 * DH : C + (h0 + 1) * DH], b_attn[C + h1 * DH : C + (h1 + 1) * DH]]),
            np.concatenate([b_attn[h2 * DH : (h2 + 1) * DH], b_attn[h3 * DH : (h3 + 1) * DH]]),
            np.concatenate([b_attn[C + h2 * DH : C + (h2 + 1) * DH], b_attn[C + h3 * DH : C + (h3 + 1) * DH]]),
        ]
    ).astype(np.float32)
    bv = np.concatenate(
        [b_attn[2 * C + h * DH : 2 * C + (h + 1) * DH] for h in heads]
    ).astype(np.float32)
    wo = np.ascontiguousarray(
        w_proj[heads[0] * DH : (heads[-1] + 1) * DH, :].astype(bf16)
    )
    mask = np.triu(np.ones((P, P))).astype(bf16)
    return {
        "xT": xT,
        "wqk": wqk,
        "wv": wv,
        "wo": wo,
        "bqk": np.ascontiguousarray(bqk),
        "bv": np.ascontiguousarray(bv),
        "mask": mask,
    }


def kernel(x, w_attn, b_attn, w_proj, b_proj):
    from concourse.bass_utils import run_bass_kernel_spmd

    x = np.asarray(x, np.float32)
    w_attn = np.asarray(w_attn, np.float32)
    b_attn = np.asarray(b_attn, np.float32)
    w_proj = np.asarray(w_proj, np.float32)
    b_proj = np.asarray(b_proj, np.float32)

    nc = get_program()
    in_maps = [
        make_core_inputs(x, w_attn, b_attn, w_proj, core) for core in range(NCORES)
    ]
    res = run_bass_kernel_spmd(nc, in_maps, core_ids=list(range(NCORES)))
    outs = [np.asarray(m["out"], np.float32) for m in res.results]

    y = np.empty((B, T, C), np.float32)
    for b in range(B):
        y[b] = outs[4 * b] + outs[4 * b + 1] + outs[4 * b + 2] + outs[4 * b + 3]
        y[b] += b_proj[None, :]
    return y


# revision 56
# speedup vs baseline: 1.1746x; 1.1746x over previous
"""Causal self-attention on 8 Trainium2 NeuronCores.

Problem: B=2, T=2048, C=1024, 16 heads x 64 dim, fp32.

Sharding: tensor-parallel over heads x data-parallel over batch.
Each core owns one batch element (cores 0-3 -> b=0, 4-7 -> b=1) and a
group of 4 consecutive heads. Each core computes:
  - QKV projection for its 4 heads (producing qT/kT transposed, V natural)
  - causal attention for its 4 heads (scores kept transposed: ST[tk, tq])
  - partial output projection (its heads' rows of w_proj)
The host sums the 4 partial projections per batch and adds b_proj.

All inputs/outputs bf16 (halves DMA; matmuls run bf16 = full PE rate).

Fused pipeline: the program is one stream per slab s of 512 queries:
  QKV(0); for s: ATT(s, pair0), ATT(s, pair1) with QKV(s+1)/OP(s-1)
  matmul units interleaved as PE fillers inside the tk loop, so the PE
  stays busy while ACT computes exp(ST).
Softmax denominators come from a ones-column appended to V (row 64 of
the PV psum accumulator).

Optimizations vs the first working kernel (166.9us -> ~152us):
  - Startup: the critical wqk(pair0-cols)/x0/wv loads stream in
    per-k-pair chunks on the SP queue so the QKV matmuls chase the
    DMA; all other loads are chained behind them with 1-element
    gpsimd dep-copies (HW DGE queues transfer concurrently and would
    otherwise halve the critical stream's bandwidth); small tensors
    ride the ACT hwdge queue.  Dummy matmuls on a scratch tile keep
    the PE busy through the load wait so the HAM clock-gate is warm
    (2.4 GHz) when real work arrives.
  - Drain: one [65,512] psum->sbuf copy per head frees the psy bank
    (hp0 on DVE, hp1 on ACT, in parallel); the reciprocal / partition-
    broadcast / normalize-multiply run as "lazy units" popped one per
    tk step inside the NEXT att call, spread so the muls pop >=2 steps
    after their gpsimd broadcasts and never head-block the strict-FIFO
    DVE queue (which previously stalled psum frees -> PE).
  - Tail: att(3,1) normalizes inline -- sm copies on ACT, per-head
    reciprocal chains on DVE, PE outer-product broadcast
    (ones[1,64].T @ rec[1,512] -> psum) instead of the ~1us gpsimd
    broadcast; dummy matmuls bridge the chain latency so HAM stays
    warm; op12's pair-0 matmuls pre-issue during the chain; tail op
    units alternate cast engines (ACT/DVE), psum rings (misc/psy) and
    DMA issue queues (SP/ACT) to avoid serializing on any one of them.
  - Explicit per-att filler lists keep every att's first PV covered
    with independent PE work (slab-0 v-blocks inside att(0,0), qk of
    the next slab inside att(s,1), op units late enough that the
    lazily-normalized yT they read is already emitted).

Device layouts (per core, DRAM):
  xT   [1024, 2048] bf16  x[b] transposed (channels on partitions)
  wqk  [1024, 512]  bf16  cols: q(h0)|q(h1)|k(h0)|k(h1)|q(h2)|q(h3)|k(h2)|k(h3)
  wv   [1024, 256]  bf16  v cols of the 4 heads
  wo   [256, 1024]  bf16  w_proj rows of the 4 heads
  bqk  [4, 128]     f32   rows: pair0-q, pair0-k, pair1-q, pair1-k biases
  bv   [256]        f32   v bias of the 4 heads
  mask [128, 128]   bf16  mask[i,j] = 1 if i<=j else 0 (tk<=tq keep)
  out  [2048, 1024] bf16  partial (pre-bias) output projection
"""

import numpy as np

B, T, C = 2, 2048, 1024
NH, DH = 16, 64
NCORES = 8
HPC = 4  # heads per core
P = 128
CK = C // P  # 8 contraction tiles over channels
NT = T // P  # 16 token tiles
SLAB = 512
NSL = T // SLAB  # 4 tq slabs

import os
WARMUP = int(os.environ.get("K_WARMUP", "52"))
ACT_DMA = int(os.environ.get("K_ACT_DMA", "1"))
FINE_LOADS = int(os.environ.get("K_FINE_LOADS", "1"))

_CACHE = {}


def _build_program():
    from contextlib import ExitStack

    import concourse.bacc as bacc
    import concourse.bass as bass
    import concourse.tile as tile
    from concourse import mybir

    f32 = mybir.dt.float32
    bf16 = mybir.dt.bfloat16
    AF = mybir.ActivationFunctionType

    nc = bacc.Bacc(
        "TRN2", target_bir_lowering=False, debug=False, num_devices=NCORES
    )

    xT = nc.dram_tensor("xT", [C, T], bf16, kind="ExternalInput").ap()
    wqk = nc.dram_tensor("wqk", [C, 4 * P], bf16, kind="ExternalInput").ap()
    wv = nc.dram_tensor("wv", [C, HPC * DH], bf16, kind="ExternalInput").ap()
    wo = nc.dram_tensor("wo", [HPC * DH, C], bf16, kind="ExternalInput").ap()
    bqk = nc.dram_tensor("bqk", [4, P], f32, kind="ExternalInput").ap()
    bv = nc.dram_tensor("bv", [HPC * DH], f32, kind="ExternalInput").ap()
    mask = nc.dram_tensor("mask", [P, P], bf16, kind="ExternalInput").ap()
    out = nc.dram_tensor("out", [T, C], bf16, kind="ExternalOutput").ap()

    with tile.TileContext(nc) as tc, ExitStack() as ctx:
        const = ctx.enter_context(tc.tile_pool(name="const", bufs=1))
        # PSUM (8 banks of [128,512]f32): pp 2x2 + psy 2x1 + misc 2x1
        ppp = ctx.enter_context(tc.tile_pool(name="ppp", bufs=2, space="PSUM"))
        psyp = ctx.enter_context(tc.tile_pool(name="psyp", bufs=2, space="PSUM"))
        miscp = ctx.enter_context(tc.tile_pool(name="miscp", bufs=2, space="PSUM"))
        expp = ctx.enter_context(tc.tile_pool(name="expp", bufs=4))
        recp = ctx.enter_context(tc.tile_pool(name="recp", bufs=4))
        outp = ctx.enter_context(tc.tile_pool(name="outp", bufs=3))

        x_sb = [
            const.tile([P, CK, SLAB], bf16, name=f"x{s}") for s in range(NSL)
        ]
        wqk_sb = const.tile([P, CK, 4 * P], bf16, name="wqk_sb")
        wv_sb = const.tile([P, CK, HPC * DH], bf16, name="wv_sb")
        wo_sb = const.tile([P, 2, C], bf16, name="wo_sb")
        bqk_sb = const.tile([P, 4], f32, name="bqk_sb")
        bv_sb = const.tile([P, HPC, DH], f32, name="bv_sb")
        mask_sb = const.tile([P, P], bf16, name="mask_sb")
        warm_sb = const.tile([P, 2 * P], bf16, name="warm_sb")
        ones_sb = const.tile([1, DH], bf16, name="ones_sb")
        v_sb = [
            const.tile([P, 4, HPC, DH + 1], bf16, name=f"v_sb{s}")
            for s in range(NSL)
        ]
        qT = [
            [const.tile([P, SLAB], bf16, name=f"qT{p}_{s}") for s in range(NSL)]
            for p in range(2)
        ]
        kT = [
            [const.tile([P, SLAB], bf16, name=f"kT{p}_{s}") for s in range(NSL)]
            for p in range(2)
        ]
        yT = [
            [const.tile([P, SLAB], bf16, name=f"yT{p}_{s}") for s in range(NSL)]
            for p in range(2)
        ]

        # --- loads -------------------------------------------------------
        # SP queue carries the startup-critical wqk/x0 stream in per-k-pair
        # chunks (the first QKV matmuls chase the DMA), then x slabs 1-3.
        # The ACT hwdge queue carries everything else in parallel.
        wqkv_ = wqk.rearrange("(k p) n -> p k n", p=P)
        xTv = xT.rearrange("(k p) t -> p k t", p=P)
        eng2 = nc.scalar if ACT_DMA else nc.sync
        if FINE_LOADS:
            # critical stream: per-k-pair chunks so QKV matmuls chase
            # DMA.  Only pair-0's wqk columns (0:256) are critical —
            # pair-1's (qk blocks 2/3) are needed ~8us later.
            for k2 in range(0, CK, 2):
                nc.sync.dma_start(
                    out=wqk_sb[:, k2 : k2 + 2, 0 : 2 * P],
                    in_=wqkv_[:, k2 : k2 + 2, 0 : 2 * P],
                )
                nc.sync.dma_start(
                    out=x_sb[0][:, k2 : k2 + 2, :],
                    in_=xTv[:, k2 : k2 + 2, 0:SLAB],
                )
        else:
            h = CK // 2
            nc.sync.dma_start(out=wqk_sb[:, :h, 0 : 2 * P], in_=wqkv_[:, :h, 0 : 2 * P])
            nc.sync.dma_start(out=x_sb[0][:, :h, :], in_=xTv[:, :h, 0:SLAB])
            nc.sync.dma_start(out=wqk_sb[:, h:, 0 : 2 * P], in_=wqkv_[:, h:, 0 : 2 * P])
            nc.sync.dma_start(out=x_sb[0][:, h:, :], in_=xTv[:, h:, 0:SLAB])
        nc.sync.dma_start(
            out=wv_sb[:], in_=wv.rearrange("(k p) n -> p k n", p=P)
        )
        # Cascade the non-critical loads: a 1-element gpsimd copy makes
        # each DMA wait for the previous one, so the startup-critical
        # wqk/x0/wv stream gets the DMA engines to itself.  (HW DGE
        # queues transfer concurrently; without this, the rest would
        # steal ~half the bandwidth exactly when QKV(0) is data-starved.)
        # The dep copies MUST NOT run on DVE/ACT: they wait on DMA
        # completion semaphores at the head of a strict-FIFO queue and
        # would block mask-muls / exps behind them.  GpSimd only carries
        # the slack-tolerant lazy broadcasts.
        def casc_dep(dst_ap, src_ap):
            nc.gpsimd.partition_broadcast(out_ap=dst_ap, in_ap=src_ap)

        casc_dep(wqk_sb[0:1, 0, 2 * P : 2 * P + 1], wv_sb[0:1, 0, 0:1])
        nc.sync.dma_start(
            out=wqk_sb[:, :, 2 * P :], in_=wqkv_[:, :, 2 * P :]
        )
        # x1 runs in parallel with wqkB (both ~0.5MB, both needed by
        # att(0,1)'s qk(1,*) fillers at about the same time)
        casc_dep(x_sb[1][0:1, 0, 0:1], wv_sb[0:1, 0, 1:2])
        h = CK // 2
        nc.sync.dma_start(out=x_sb[1][:, :h, :], in_=xTv[:, :h, SLAB : 2 * SLAB])
        nc.sync.dma_start(out=x_sb[1][:, h:, :], in_=xTv[:, h:, SLAB : 2 * SLAB])
        cascade = [x_sb[2], wo_sb, x_sb[3]]
        prev_dep = x_sb[1][0:1, CK - 1, SLAB - 1 : SLAB]
        for tgt in cascade:
            if tgt is wo_sb:
                casc_dep(wo_sb[0:1, 0, 0:1], prev_dep)
                eng2.dma_start(
                    out=wo_sb[:], in_=wo.rearrange("(r p) n -> p r n", p=P)
                )
                prev_dep = wo_sb[0:1, 1, C - 1 : C]
            else:
                s = x_sb.index(tgt)
                h = CK // 2
                casc_dep(tgt[0:1, 0, 0:1], prev_dep)
                nc.sync.dma_start(
                    out=tgt[:, :h, :],
                    in_=xTv[:, :h, s * SLAB : (s + 1) * SLAB],
                )
                nc.sync.dma_start(
                    out=tgt[:, h:, :],
                    in_=xTv[:, h:, s * SLAB : (s + 1) * SLAB],
                )
                prev_dep = tgt[0:1, CK - 1, SLAB - 1 : SLAB]
        # small/late tensors on the ACT hwdge queue (ACT boots ~7us; these
        # must not steal DMA bandwidth from the critical wqk/x0 stream)
        eng2.dma_start(out=bqk_sb[:], in_=bqk.rearrange("r p -> p r"))
        eng2.dma_start(out=mask_sb[:], in_=mask)
        bv_bcast = bass.AP(
            tensor=bv.tensor,
            offset=bv.offset,
            ap=[[0, P], *bv.rearrange("(h d) -> h d", d=DH).ap],
        )
        eng2.dma_start(out=bv_sb[:], in_=bv_bcast)
        for s in range(NSL):
            nc.vector.memset(v_sb[s][:, :, :, DH : DH + 1], 1.0)
        nc.vector.memset(warm_sb[:], 0.125)
        nc.vector.memset(ones_sb[:], 1.0)

        # --- PE warmup: dummy matmuls on the scratch tile keep the PE
        # busy through the load wait so HAM un-throttles (K=8/8) before
        # the first real matmul (~3.4us of sustained activity needed).
        if WARMUP:
            wps = miscp.tile([P, SLAB], f32, name="wps", tag="m")
            for _ in range(WARMUP):
                nc.tensor.matmul(
                    wps[:, 0:P],
                    lhsT=warm_sb[:, 0:P],
                    rhs=warm_sb[:, P : 2 * P],
                    start=True,
                    stop=True,
                )

        # --- work units ---
        def qk_block(s, blk):
            """One q/k column block of QKV(s): 8 chained MMs + ACT bias."""
            p, qk = divmod(blk, 2)
            dst = qT[p][s] if qk == 0 else kT[p][s]
            ps = miscp.tile([P, SLAB], f32, name="ps_qkv", tag="m")
            for k in range(CK):
                nc.tensor.matmul(
                    ps[:],
                    lhsT=wqk_sb[:, k, blk * P : (blk + 1) * P],
                    rhs=x_sb[s][:, k, :],
                    start=(k == 0),
                    stop=(k == CK - 1),
                )
            nc.scalar.activation(
                out=dst[:],
                in_=ps[:],
                func=AF.Identity,
                bias=bqk_sb[:, blk : blk + 1],
                scale=1.0,
            )

        def v_block(s, tt):
            """V for token tile 4s+tt: 8 chained MMs + DVE bias add."""
            ps = miscp.tile([P, SLAB], f32, name="ps_v", tag="m")
            for k in range(CK):
                nc.tensor.matmul(
                    ps[:, : HPC * DH],
                    lhsT=x_sb[s][:, k, tt * P : (tt + 1) * P],
                    rhs=wv_sb[:, k, :],
                    start=(k == 0),
                    stop=(k == CK - 1),
                )
            nc.vector.tensor_add(
                out=v_sb[s][:, tt, :, 0:DH],
                in0=ps[:, : HPC * DH].rearrange("p (h d) -> p h d", d=DH),
                in1=bv_sb[:],
            )

        def op_unit(t, act_cast=False, split_dma=False, psy_pool=False):
            """Output projection for token tile t + drain + DMA."""
            ob = outp.tile([P, C], bf16, name="ob", tag="ob")
            for ns in range(2):
                if psy_pool:  # tail only: psy banks are free, doubles
                    ps = psyp.tile([P, SLAB], f32, name="psot", tag="psy")
                else:
                    ps = miscp.tile([P, SLAB], f32, name="pso", tag="m")
                for p in range(2):
                    nc.tensor.matmul(
                        ps[:],
                        lhsT=yT[p][t // 4][:, (t % 4) * P : (t % 4 + 1) * P],
                        rhs=wo_sb[:, p, ns * SLAB : (ns + 1) * SLAB],
                        start=(p == 0),
                        stop=(p == 1),
                    )
                if act_cast:
                    nc.scalar.copy(
                        out=ob[:, ns * SLAB : (ns + 1) * SLAB], in_=ps[:]
                    )
                else:
                    nc.vector.tensor_copy(
                        out=ob[:, ns * SLAB : (ns + 1) * SLAB], in_=ps[:]
                    )
                # tail ops split DMA issue across SP/ACT queues (12 back
                # -to-back issues at 565ns each would serialize ~7us)
                deng = nc.scalar if (split_dma and ns == 1) else nc.sync
                deng.dma_start(
                    out=out[t * P : (t + 1) * P, ns * SLAB : (ns + 1) * SLAB],
                    in_=ob[:, ns * SLAB : (ns + 1) * SLAB],
                )

        lazy = []  # deferred normalize sub-units, popped 1/tk-step
        noop_u = lambda: None

        def att(s, p, fillers, post=(), last=False):
            """Causal attention for head pair p over tq slab s.

            Pops one PE filler + one lazy normalize unit per tk step.
            """
            ntk = 4 * s + 4
            psy = [
                psyp.tile([P, SLAB], f32, name=f"psy{hp}", tag="psy")
                for hp in range(2)
            ]

            def off_of(tk):
                d = tk - 4 * s
                return d * P if d >= 0 else 0

            pend = {}
            exd = {}

            def st(tk):
                off = off_of(tk)
                pp = ppp.tile([P, 2 * SLAB], f32, name="pp", tag="pp")
                for hp in range(2):
                    nc.tensor.matmul(
                        pp[:, hp * SLAB + off : (hp + 1) * SLAB],
                        lhsT=kT[p][tk // 4][
                            hp * DH : (hp + 1) * DH,
                            (tk % 4) * P : (tk % 4 + 1) * P,
                        ],
                        rhs=qT[p][s][hp * DH : (hp + 1) * DH, off:],
                        start=True,
                        stop=True,
                    )
                pend[tk] = pp

            def do_exp(tk):
                off = off_of(tk)
                pp = pend.pop(tk)
                ex = expp.tile([P, 2 * SLAB], bf16, name="ex", tag="ex")
                ppv = pp[:].rearrange("q (h n) -> q h n", h=2)[:, :, off:]
                exv = ex[:].rearrange("q (h n) -> q h n", h=2)[:, :, off:]
                nc.scalar.activation(
                    out=exv,
                    in_=ppv,
                    func=AF.Exp,
                    scale=float(1.0 / np.sqrt(DH)),
                )
                if tk - 4 * s >= 0:
                    exm = ex[:].rearrange("q (h n) -> q h n", h=2)[
                        :, :, off : off + P
                    ]
                    mask2 = bass.AP(
                        tensor=mask_sb[:].tensor,
                        offset=mask_sb[:].offset,
                        ap=[mask_sb[:].ap[0], [0, 2], mask_sb[:].ap[1]],
                    )
                    nc.vector.tensor_mul(out=exm, in0=exm, in1=mask2)
                exd[tk] = ex

            st(0)
            if ntk > 1:
                st(1)
            do_exp(0)
            for tk in range(ntk):
                off = off_of(tk)
                if tk + 2 < ntk:
                    st(tk + 2)
                if lazy:
                    lazy.pop(0)()
                if fillers:
                    fillers.pop(0)()
                if tk + 1 < ntk:
                    do_exp(tk + 1)
                ex = exd.pop(tk)
                for hp in range(2):
                    nc.tensor.matmul(
                        psy[hp][0 : DH + 1, off:],
                        lhsT=v_sb[tk // 4][:, tk % 4, 2 * p + hp, :],
                        rhs=ex[:, hp * SLAB + off : (hp + 1) * SLAB],
                        start=(tk == 0),
                        stop=(tk == ntk - 1),
                    )
            if last:
                # tail fast path.  The denominator rows are pulled from
                # PSUM immediately (sm copies), reciprocals cast to bf16
                # feed a PE outer-product broadcast (ones[1,64].T @ rec)
                # while dummy+post matmuls keep the PE warm; normalize
                # muls land before post[1] so the final OP tiles start
                # right after.  Post casts go to ACT (DVE is the tail
                # critical path).
                post = list(post)
                # dummies bridge PE to the post matmuls (whose psum
                # slots wait on casts); pp ring is free here
                wps2 = ppp.tile([P, 2 * SLAB], f32, name="wps2", tag="pp")
                for _ in range(16):
                    nc.tensor.matmul(
                        wps2[:, 0 : 2 * P],
                        lhsT=warm_sb[:, 0:P],
                        rhs=warm_sb[:],
                        start=True,
                        stop=True,
                    )
                # per-head chains: sm on ACT, recip+cast on DVE, so the
                # hp0 broadcast matmul can start ~3 ops after last PV
                recs = []
                for hp in range(2):
                    sm = recp.tile([1, SLAB], f32, name="smL", tag="smL")
                    nc.scalar.copy(out=sm[:], in_=psy[hp][DH : DH + 1, :])
                    recf = recp.tile([1, SLAB], f32, name="recf", tag="recf")
                    nc.vector.reciprocal_approx_fast(
                        out=recf[:], in_=sm[:]
                    )
                    rec = recp.tile([1, SLAB], bf16, name="recb", tag="recb")
                    nc.vector.tensor_copy(out=rec[:], in_=recf[:])
                    recs.append(rec)
                yfull = []
                for hp in range(2):
                    yf = recp.tile([DH, SLAB], f32, name="yfL", tag="yfL")
                    if hp == 0:
                        nc.vector.tensor_copy(out=yf[:], in_=psy[hp][0:DH, :])
                    else:
                        nc.scalar.copy(out=yf[:], in_=psy[hp][0:DH, :])
                    yfull.append(yf)
                # post MMs cover the DVE chain; second unit on the psy
                # ring (free after the sm/yf reads) so the two units
                # don't serialize on the 2-slot misc ring
                if post:
                    post[0](act_cast=True, split_dma=True)
                for f in post[1:]:
                    f(act_cast=True, split_dma=True, psy_pool=True)
                # pre-issue op12's pair-0 matmuls (yT[0][3] is already
                # normalized via the lazy units) during the chain wait
                pre_ps = []
                for ns in range(2):
                    ps = miscp.tile([P, SLAB], f32, name="pso", tag="m")
                    nc.tensor.matmul(
                        ps[:],
                        lhsT=yT[0][3][:, 0:P],
                        rhs=wo_sb[:, 0, ns * SLAB : (ns + 1) * SLAB],
                        start=True,
                        stop=False,
                    )
                    pre_ps.append(ps)
                rbs = []
                for hp in range(2):
                    rb = ppp.tile([P, 2 * SLAB], f32, name="rbL", tag="pp")
                    nc.tensor.matmul(
                        rb[0:DH, 0:SLAB],
                        lhsT=ones_sb[:],
                        rhs=recs[hp][:],
                        start=True,
                        stop=True,
                    )
                    rbs.append(rb)
                for hp in range(2):
                    nc.vector.tensor_mul(
                        out=yT[p][s][hp * DH : (hp + 1) * DH, :],
                        in0=yfull[hp][:],
                        in1=rbs[hp][0:DH, 0:SLAB],
                    )
                # finish op12: pair-1 matmuls + drains
                ob = outp.tile([P, C], bf16, name="ob", tag="ob")
                for ns in range(2):
                    nc.tensor.matmul(
                        pre_ps[ns][:],
                        lhsT=yT[1][3][:, 0:P],
                        rhs=wo_sb[:, 1, ns * SLAB : (ns + 1) * SLAB],
                        start=False,
                        stop=True,
                    )
                    if ns == 0:
                        nc.scalar.copy(
                            out=ob[:, ns * SLAB : (ns + 1) * SLAB],
                            in_=pre_ps[ns][:],
                        )
                    else:
                        nc.vector.tensor_copy(
                            out=ob[:, ns * SLAB : (ns + 1) * SLAB],
                            in_=pre_ps[ns][:],
                        )
                    deng = nc.scalar if ns == 1 else nc.sync
                    deng.dma_start(
                        out=out[12 * P : 13 * P, ns * SLAB : (ns + 1) * SLAB],
                        in_=ob[:, ns * SLAB : (ns + 1) * SLAB],
                    )
                return
            # drain: ONE [65,512] copy per head frees the psy bank fast
            # (hp0 on DVE, hp1 on ACT so they land in parallel).
            yfull = []
            for hp in range(2):
                yf = recp.tile([DH + 1, SLAB], f32, name="yf", tag="yf")
                if hp == 0:
                    nc.vector.tensor_copy(out=yf[:], in_=psy[hp][0 : DH + 1, :])
                else:
                    nc.scalar.copy(out=yf[:], in_=psy[hp][0 : DH + 1, :])
                yfull.append(yf)
            for f in post:  # PE work emitted after the drain copies
                f()
            if True:
                # deferred: recip+bcast then mul, popped in the next att
                def u_rb(hp):
                    def run():
                        # copy row 64 to partition 0 first: DVE table ops
                        # (recip) cannot shift partitions on HW
                        sm = recp.tile([1, SLAB], f32, name="sm", tag="sm")
                        nc.vector.tensor_copy(
                            out=sm[:], in_=yfull[hp][DH : DH + 1, :]
                        )
                        rec = recp.tile([1, SLAB], f32, name="rec", tag="rec")
                        nc.vector.reciprocal_approx_fast(
                            out=rec[:], in_=sm[:]
                        )
                        rb = recp.tile([DH, SLAB], f32, name="rb", tag="rb")
                        nc.gpsimd.partition_broadcast(
                            out_ap=rb[:], in_ap=rec[:]
                        )
                        yfull.append(rb)  # stash for u_mul

                    return run

                def u_mul(hp):
                    def run():
                        rb = yfull[2 + hp]
                        nc.vector.tensor_mul(
                            out=yT[p][s][hp * DH : (hp + 1) * DH, :],
                            in0=yfull[hp][0:DH, :],
                            in1=rb[:],
                        )

                    return run

                # spread pops: muls land >=2 steps after their gpsimd
                # broadcasts so they never head-block the DVE FIFO
                lazy.extend(
                    [u_rb(0), u_rb(1), noop_u, u_mul(0), u_mul(1), noop_u]
                )

        # --- fused schedule (explicit per-att filler lists) ---
        # Constraints encoded below:
        #  - v(0,*) interleave INTO att(0,0) so attention starts as soon
        #    as qk(0,0/1) + v(0,0) land (startup is DMA-paced).
        #  - qk(s,2/3) must be emitted before att(s,1) starts (st reads
        #    them), so qk(0,3) is emitted between the slab-0 atts.
        #  - every att(s,1) list starts with real PE work so its first
        #    PV (waiting on exp(0)) never idles the PE.
        #  - op units for slab s-1 pop only at step >= 4 of att(s,0)
        #    (the lazy normalize muls for yT[1][s-1] pop at steps 2-3).
        def qk(s, b):
            return lambda: qk_block(s, b)

        def vb(s, t):
            return lambda: v_block(s, t)

        def op(t):
            return lambda **kw: op_unit(t, **kw)

        noop = lambda: None
        # blocks 0+1 fused with interleaved chains (2 psum banks): each
        # arriving wqk/x0 DMA chunk-pair feeds 4 matmuls, matching the
        # chunk cadence so the DMA-paced startup has no PE idle
        ps01 = [
            miscp.tile([P, SLAB], f32, name=f"ps_qkv{b}", tag="m")
            for b in range(2)
        ]
        for k in range(CK):
            for b in range(2):
                nc.tensor.matmul(
                    ps01[b][:],
                    lhsT=wqk_sb[:, k, b * P : (b + 1) * P],
                    rhs=x_sb[0][:, k, :],
                    start=(k == 0),
                    stop=(k == CK - 1),
                )
        for b, dst in ((0, qT[0][0]), (1, kT[0][0])):
            nc.scalar.activation(
                out=dst[:],
                in_=ps01[b][:],
                func=AF.Identity,
                bias=bqk_sb[:, b : b + 1],
                scale=1.0,
            )
        v_block(0, 0)
        att(0, 0, [vb(0, 1), vb(0, 2), vb(0, 3), qk(0, 2)])
        qk_block(0, 3)
        att(0, 1, [qk(1, 0), qk(1, 1), qk(1, 2), qk(1, 3)])
        att(1, 0, [vb(1, 0), vb(1, 1), vb(1, 2), vb(1, 3), noop, noop,
                   op(0), op(1)])
        att(1, 1, [qk(2, 0), qk(2, 1), qk(2, 2), qk(2, 3), vb(2, 0),
                   vb(2, 1)], post=[op(2), op(3)])
        att(2, 0, [vb(2, 2), vb(2, 3), noop, noop, noop, noop, op(4), op(5)])
        att(2, 1, [qk(3, 0), qk(3, 1), qk(3, 2), qk(3, 3), vb(3, 0),
                   vb(3, 1)], post=[op(6), op(7)])
        att(3, 0, [vb(3, 2), vb(3, 3), noop, noop, noop, noop, noop, op(8)])
        att(3, 1, [op(9)], post=[op(10), op(11)], last=True)  # op12 inlined
        for t in range(13, NT):
            op_unit(
                t,
                act_cast=(t % 2 == 0),
                split_dma=True,
                psy_pool=(t % 2 == 1),
            )

    nc.compile()
    return nc


def get_program():
    if "nc" not in _CACHE:
        _CACHE["nc"] = _build_program()
    return _CACHE["nc"]


def make_core_inputs(x, w_attn, b_attn, w_proj, core):
    """Host-side shard preparation for one core."""
    import ml_dtypes

    bf16 = ml_dtypes.bfloat16
    b = core // 4
    g = core % 4
    heads = [4 * g + i for i in range(HPC)]

    xT = np.ascontiguousarray(np.asarray(x[b], np.float32).T.astype(bf16))

    def qcols(h):
        return w_attn[:, h * DH : (h + 1) * DH]

    def kcols(h):
        return w_attn[:, C + h * DH : C + (h + 1) * DH]

    def vcols(h):
        return w_attn[:, 2 * C + h * DH : 2 * C + (h + 1) * DH]

    h0, h1, h2, h3 = heads
    wqk = np.ascontiguousarray(
        np.concatenate(
            [qcols(h0), qcols(h1), kcols(h0), kcols(h1),
             qcols(h2), qcols(h3), kcols(h2), kcols(h3)],
            axis=1,
        ).astype(bf16)
    )
    wv = np.ascontiguousarray(
        np.concatenate([vcols(h) for h in heads], axis=1).astype(bf16)
    )
    bqk = np.stack(
        [
            np.concatenate([b_attn[h0 * DH : (h0 + 1) * DH], b_attn[h1 * DH : (h1 + 1) * DH]]),
            np.concatenate([b_attn[C + h0 * DH : C + (h0 + 1) * DH], b_attn[C + h1 * DH : C + (h1 + 1) * DH]]),
            np.concatenate([b_attn[h2 * DH : (h2 + 1) * DH], b_attn[h3 * DH : (h3 + 1) * DH]]),
            np.concatenate([b_attn[C + h2 * DH : C + (h2 + 1) * DH], b_attn[C + h3 * DH : C + (h3 + 1) * DH]]),
        ]
    ).astype(np.float32)
    bv = np.concatenate(
        [b_attn[2 * C + h * DH : 2 * C + (h + 1) * DH] for h in heads]
    ).astype(np.float32)
    wo = np.ascontiguousarray(
        w_proj[heads[0] * DH : (heads[-1] + 1) * DH, :].astype(bf16)
    )
    mask = np.triu(np.ones((P, P))).astype(bf16)
    return {
        "xT": xT,
        "wqk": wqk,
        "wv": wv,
        "wo": wo,
        "bqk": np.ascontiguousarray(bqk),
        "bv": np.ascontiguousarray(bv),
        "mask": mask,
    }


def kernel(x, w_attn, b_attn, w_proj, b_proj):
    from concourse.bass_utils import run_bass_kernel_spmd

    x = np.asarray(x, np.float32)
    w_attn = np.asarray(w_attn, np.float32)
    b_attn = np.asarray(b_attn, np.float32)
    w_proj = np.asarray(w_proj, np.float32)
    b_proj = np.asarray(b_proj, np.float32)

    nc = get_program()
    in_maps = [
        make_core_inputs(x, w_attn, b_attn, w_proj, core) for core in range(NCORES)
    ]
    res = run_bass_kernel_spmd(nc, in_maps, core_ids=list(range(NCORES)))
    outs = [np.asarray(m["out"], np.float32) for m in res.results]

    y = np.empty((B, T, C), np.float32)
    for b in range(B):
        y[b] = outs[4 * b] + outs[4 * b + 1] + outs[4 * b + 2] + outs[4 * b + 3]
        y[b] += b_proj[None, :]
    return y
